# revision 15
# baseline (speedup 1.0000x reference)
"""CompositionalAttention TRN2 kernel.

Full (unsharded) inputs in, full output out.  Internally: 8 NeuronCores,
data-parallel over batch (4 cores per batch element) x parallel over query
rows (512 rows per core, all 8 search heads per core).

The axon tunnel to the cores is ~45 MB/s, so the kernel minimizes
host<->device traffic: each core uploads only a unique 1/8 shard of the
problem (its query-block of x^T in fp16, a 1/8 row-shard of each weight in
fp16) and the full per-batch data is reconstructed ON DEVICE with
AllGather collectives (NeuronLink is fast).  Output is downloaded as fp16.
The PJRT executable is built once and cached; output buffers are recycled
as donated inputs so no zero-buffer upload is paid per call.

Math (per batch b, search head s, query row i):
  sq = (x @ Wsq) * sc ; sk = x @ Wsk          (per head, d=64)
  P  = softmax_j(sq_i . sk_j)                 (n x n attention)
  U_r = P @ rv_r                              (rv = x @ Wrv, r=0,1)
  retrieved_r = U_r / l,  l = sum_j expP
  sim_r = rq . (retrieved_r @ Wrk) = rowdot(U_r, rq @ Wrk^T) / l
  attn = softmax_r(sim)  ==  sigmoid(sim_0 - sim_1) for r=2 (exact)
  out_s = attn*retrieved_0 + (1-attn)*retrieved_1
  out = concat_s(out_s) @ Wout

Host folds: scale into Wsq; Wrk into Wrq (rqW = x @ (sc * Wrq_s @ Wrk^T));
mask into an additive exp bias.  exp is computed without max-subtraction
(sim ~ N(0,1), max |sim| << 80, so fp16 exp inputs are safe).
"""

import sys

sys.path.insert(0, "/opt/trn_rl_repo")

import numpy as np

B, N, DIM, S, R, DH = 2, 2048, 1024, 8, 2, 64
SD, RD = S * DH, R * DH  # 512, 128
NCORES = 8
NSLICE = N // 4  # 512 query rows per core
SCALE = DH**-0.5
KT = DIM // 128  # 8 contraction tiles
JT = N // 128  # 16 key tiles
ICN = NSLICE // 128  # 4 query chunks
PAIRS = S // 2
WSHARD = DIM // NCORES  # 128 rows of each [DIM, .] weight per core
WOSHARD = SD // NCORES  # 64 rows of Wout per core

_cache = {}


def _build_program():
    import concourse.bass as bass
    import concourse.tile as tile
    from concourse import bacc, mybir
    from concourse.masks import make_identity

    f32 = mybir.dt.float32
    f16 = mybir.dt.float16
    u8 = mybir.dt.uint8
    Exp = mybir.ActivationFunctionType.Exp
    Sigmoid = mybir.ActivationFunctionType.Sigmoid
    add = mybir.AluOpType.add
    mult = mybir.AluOpType.mult
    band = mybir.AluOpType.bitwise_and
    shr = mybir.AluOpType.logical_shift_right

    nc = bacc.Bacc(
        "TRN2", target_bir_lowering=False, debug=False, num_devices=NCORES
    )

    # Per-core unique inputs (x packed to 12 bits, weights fp16, to cut
    # tunnel bytes).
    # xq: this core's query block of x^T, i.e. xT[:, isl*512:(isl+1)*512],
    # quantized to 12-bit fixed point and packed as 3 byte-planes
    # [A | B | C] where token u pairs with token u+256:
    #   v0 = A + 256*(B & 15),  v1 = C + 256*(B >> 4),  x = (v - 2048)/2048
    # (the dynamic quantization scale is folded into the weights on host).
    PACK = 3 * NSLICE // 2  # 768 packed bytes per row
    xqd = nc.dram_tensor("xq", [DIM, PACK], u8, kind="ExternalInput").ap()
    mbd = nc.dram_tensor("mb", [N], f32, kind="ExternalInput").ap()
    wsqd = nc.dram_tensor("wsq", [WSHARD, SD], f16, kind="ExternalInput").ap()
    wskd = nc.dram_tensor("wsk", [WSHARD, SD], f16, kind="ExternalInput").ap()
    wrqd = nc.dram_tensor("wrq", [WSHARD, SD], f16, kind="ExternalInput").ap()
    wrvd = nc.dram_tensor("wrv", [WSHARD, RD], f16, kind="ExternalInput").ap()
    woutd = nc.dram_tensor("wout", [WOSHARD, DIM], f16, kind="ExternalInput").ap()
    # Output is packed the same way as x: 12-bit planar [A | B | C] along
    # columns (v0 = cols 0:512, v1 = cols 512:1024 of the f32 output), with
    # a per-core dynamic scale in osc: out = (v - 2048) * osc.
    OPACK = 3 * DIM // 2  # 1536 packed bytes per row
    outd = nc.dram_tensor("out", [NSLICE, OPACK], u8, kind="ExternalOutput").ap()
    oscd = nc.dram_tensor("osc", [1, 1], f32, kind="ExternalOutput").ap()

    GROUPS_X = [[0, 1, 2, 3], [4, 5, 6, 7]]
    GROUPS_W = [list(range(NCORES))]

    with tile.TileContext(nc) as tc:
        with (
            tc.tile_pool(name="gdram", bufs=16, space="DRAM") as gdram,
            tc.tile_pool(name="sk", bufs=4) as skp,
            tc.tile_pool(name="sq", bufs=4) as sqp,
            tc.tile_pool(name="rqw", bufs=4) as rqwp,
            tc.tile_pool(name="rvaug", bufs=JT) as rvap,
            tc.tile_pool(name="consts", bufs=4) as constp,
            tc.tile_pool(name="outcat", bufs=4) as outcatp,
            tc.tile_pool(name="woutp", bufs=4) as woutp,
            tc.tile_pool(name="psA", bufs=2, space="PSUM") as psA,
        ):
            # ============ Phase 0: on-device allgather of shards ============
            # x: gather the 4 query-blocks of this batch group (still packed).
            # xg layout: block g rows [g*1024,(g+1)*1024) = packed
            # xT[:, g*512:(g+1)*512]
            xq_b = gdram.tile([DIM, PACK], u8, name="xq_b")
            xg = gdram.tile([4 * DIM, PACK], u8, name="xg")
            nc.gpsimd.dma_start(xq_b[:], xqd[:])
            nc.gpsimd.collective_compute(
                "AllGather",
                mybir.AluOpType.bypass,
                replica_groups=GROUPS_X,
                ins=[xq_b.opt()],
                outs=[xg.opt()],
            )

            def gather_w(name, ind, shard_rows, cols):
                b = gdram.tile([shard_rows, cols], f16, name=name + "_b")
                g = gdram.tile(
                    [NCORES * shard_rows, cols], f16, name=name + "_g",
                    addr_space="Shared",
                )
                nc.gpsimd.dma_start(b[:], ind[:])
                nc.gpsimd.collective_compute(
                    "AllGather",
                    mybir.AluOpType.bypass,
                    replica_groups=GROUPS_W,
                    ins=[b.opt()],
                    outs=[g.opt()],
                )
                return g

            wsq_g = gather_w("wsq", wsqd, WSHARD, SD)
            wsk_g = gather_w("wsk", wskd, WSHARD, SD)
            wrq_g = gather_w("wrq", wrqd, WSHARD, SD)
            wrv_g = gather_w("wrv", wrvd, WSHARD, RD)
            wout_g = gather_w("wout", woutd, WOSHARD, DIM)

            # --- constants ---
            mb = constp.tile([128, JT], f32, tag="mb", name="mb")
            nc.sync.dma_start(mb[:], mbd.rearrange("(t p) -> p t", p=128))
            identity = constp.tile([128, 128], f32, tag="ident", name="ident")
            make_identity(nc, identity[:])

            skT = [skp.tile([128, N], f16, tag="skT", name="skT") for _ in range(4)]
            sqT = [sqp.tile([128, NSLICE], f16, tag="sqT", name="sqT") for _ in range(4)]
            rqW = [rqwp.tile([128, SD], f32, tag="rqW", name="rqW") for _ in range(4)]
            rvaug = [rvap.tile([128, 132], f16, tag="rvaug", name="rvaug") for _ in range(JT)]

            # ============ Phase 1: projections ============
            with (
                tc.tile_pool(name="xt", bufs=KT) as xtp,
                tc.tile_pool(name="xtq", bufs=KT) as xtqp,
                tc.tile_pool(name="unp", bufs=10) as unp,
                tc.tile_pool(name="wl", bufs=12) as wlp,
                tc.tile_pool(name="wrq", bufs=KT) as wrqp,
                tc.tile_pool(name="rvbf", bufs=1) as rvbfp,
            ):
                H = NSLICE // 2  # 256 tokens per unpacked plane

                def unpack_block(dst, dst_col, src, src_row):
                    # src rows [src_row, src_row+128) hold a packed
                    # [128, 768] block; writes 512 f16 tokens at
                    # dst[:, dst_col : dst_col+512].
                    P = unp.tile([128, PACK], u8, tag="P", name="P")
                    nc.sync.dma_start(P[:], src[src_row : src_row + 128, :])
                    af = unp.tile([128, H], f32, tag="af", name="af")
                    nc.vector.tensor_copy(af[:], P[:, 0:H])
                    cf = unp.tile([128, H], f32, tag="cf", name="cf")
                    nc.vector.tensor_copy(cf[:], P[:, 2 * H : 3 * H])
                    b15u = unp.tile([128, H], u8, tag="b15u", name="b15u")
                    nc.vector.tensor_scalar(b15u[:], P[:, H : 2 * H], 15, None, op0=band)
                    b15 = unp.tile([128, H], f32, tag="b15", name="b15")
                    nc.vector.tensor_copy(b15[:], b15u[:])
                    bhiu = unp.tile([128, H], u8, tag="bhiu", name="bhiu")
                    nc.vector.tensor_scalar(bhiu[:], P[:, H : 2 * H], 4, None, op0=shr)
                    bhi = unp.tile([128, H], f32, tag="bhi", name="bhi")
                    nc.vector.tensor_copy(bhi[:], bhiu[:])
                    v0 = unp.tile([128, H], f32, tag="v0", name="v0")
                    nc.vector.tensor_scalar(v0[:], b15[:], 256.0, None, op0=mult)
                    nc.vector.tensor_add(v0[:], v0[:], af[:])
                    v1 = unp.tile([128, H], f32, tag="v1", name="v1")
                    nc.vector.tensor_scalar(v1[:], bhi[:], 256.0, None, op0=mult)
                    nc.vector.tensor_add(v1[:], v1[:], cf[:])
                    nc.vector.tensor_scalar(
                        dst[:, dst_col : dst_col + H],
                        v0[:], -2048.0, 1.0 / 2048.0, op0=add, op1=mult,
                    )
                    nc.vector.tensor_scalar(
                        dst[:, dst_col + H : dst_col + 2 * H],
                        v1[:], -2048.0, 1.0 / 2048.0, op0=add, op1=mult,
                    )

                xt = []
                xtq = []
                for kt in range(KT):
                    t = xtp.tile([128, N], f16, tag="xt", name="xt")
                    for g in range(4):
                        unpack_block(t, g * NSLICE, xg, g * DIM + kt * 128)
                    xt.append(t)
                    tq = xtqp.tile([128, NSLICE], f16, tag="xtq", name="xtq")
                    unpack_block(tq, 0, xqd, kt * 128)
                    xtq.append(tq)
                wrqt = []
                for kt in range(KT):
                    t = wrqp.tile([128, SD], f16, tag="wrq", name="wrq")
                    nc.sync.dma_start(t[:], wrq_g[kt * 128 : (kt + 1) * 128, :])
                    wrqt.append(t)

                # skT[dt] = (Wsk[:, dt]).T-proj of x: [128 d, 2048 j]
                for dt in range(4):
                    wk = []
                    for kt in range(KT):
                        t = wlp.tile([128, 128], f16, tag="wl", name="wl")
                        nc.sync.dma_start(
                            t[:],
                            wsk_g[kt * 128 : (kt + 1) * 128, dt * 128 : (dt + 1) * 128],
                        )
                        wk.append(t)
                    for jc in range(4):
                        ps = psA.tile([128, 512], f32, tag="psA", name="psA")
                        for kt in range(KT):
                            nc.tensor.matmul(
                                ps[:],
                                wk[kt][:],
                                xt[kt][:, jc * 512 : (jc + 1) * 512],
                                start=(kt == 0),
                                stop=(kt == KT - 1),
                            )
                        nc.vector.tensor_copy(
                            skT[dt][:, jc * 512 : (jc + 1) * 512], ps[:]
                        )

                # sqT[dt]: [128 d, 512 i] (scale pre-folded into Wsq)
                for dt in range(4):
                    wk = []
                    for kt in range(KT):
                        t = wlp.tile([128, 128], f16, tag="wl", name="wl")
                        nc.sync.dma_start(
                            t[:],
                            wsq_g[kt * 128 : (kt + 1) * 128, dt * 128 : (dt + 1) * 128],
                        )
                        wk.append(t)
                    ps = psA.tile([128, 512], f32, tag="psA", name="psA")
                    for kt in range(KT):
                        nc.tensor.matmul(
                            ps[:],
                            wk[kt][:],
                            xtq[kt][:],
                            start=(kt == 0),
                            stop=(kt == KT - 1),
                        )
                    nc.vector.tensor_copy(sqT[dt][:], ps[:])

                # rqW[ic]: row-land [128 i, 512 sd] = x_i @ (sc*Wrq_s@Wrk^T)
                for ic in range(ICN):
                    ps = psA.tile([128, 512], f32, tag="psA", name="psA")
                    for kt in range(KT):
                        nc.tensor.matmul(
                            ps[:],
                            xtq[kt][:, ic * 128 : (ic + 1) * 128],
                            wrqt[kt][:],
                            start=(kt == 0),
                            stop=(kt == KT - 1),
                        )
                    nc.vector.tensor_copy(rqW[ic][:], ps[:])

                # rvT [128 d, 2048 j] -> transpose to rv_aug [j, 132] (f16)
                rvbf = rvbfp.tile([128, N], f32, tag="rvbf", name="rvbf")
                wrvt = []
                for kt in range(KT):
                    t = wlp.tile([128, 128], f16, tag="wl", name="wl")
                    nc.sync.dma_start(t[:], wrv_g[kt * 128 : (kt + 1) * 128, :])
                    wrvt.append(t)
                for jc in range(4):
                    ps = psA.tile([128, 512], f32, tag="psA", name="psA")
                    for kt in range(KT):
                        nc.tensor.matmul(
                            ps[:],
                            wrvt[kt][:],
                            xt[kt][:, jc * 512 : (jc + 1) * 512],
                            start=(kt == 0),
                            stop=(kt == KT - 1),
                        )
                    nc.vector.tensor_copy(rvbf[:, jc * 512 : (jc + 1) * 512], ps[:])
                for jt in range(JT):
                    nc.gpsimd.memset(rvaug[jt][:], 1.0)
                for g in range(4):
                    ps = psA.tile([128, 512], f32, tag="psA", name="psA")
                    for k in range(4):
                        jt = g * 4 + k
                        nc.tensor.transpose(
                            ps[:, k * 128 : (k + 1) * 128],
                            rvbf[:, jt * 128 : (jt + 1) * 128],
                            identity[:],
                        )
                    for k in range(4):
                        jt = g * 4 + k
                        nc.vector.tensor_copy(
                            rvaug[jt][:, 0:128], ps[:, k * 128 : (k + 1) * 128]
                        )

            # ============ Phase 2: attention + retrieval ============
            woutt = []
            for sc in range(4):
                t = woutp.tile([128, DIM], f16, tag="wout", name="wout")
                nc.sync.dma_start(t[:], wout_g[sc * 128 : (sc + 1) * 128, :])
                woutt.append(t)

            outcat = [outcatp.tile([128, SD], f32, tag="outcat", name="outcat") for _ in range(4)]

            with (
                tc.tile_pool(name="expp", bufs=36) as expp,
                tc.tile_pool(name="small", bufs=16) as smallp,
                tc.tile_pool(name="scr", bufs=4) as scrp,
                tc.tile_pool(name="psQK", bufs=2, space="PSUM") as psQK,
                tc.tile_pool(name="psU", bufs=4, space="PSUM") as psU,
            ):
                for p in range(PAIRS):
                    expP = [[None] * JT, [None] * JT]
                    for jt in range(JT):
                        for h in range(2):
                            qk = psQK.tile([128, 512], f32, tag="qk", name="qk")
                            lo, hi = h * 64, (h + 1) * 64
                            nc.tensor.matmul(
                                qk[:],
                                skT[p][lo:hi, jt * 128 : (jt + 1) * 128],
                                sqT[p][lo:hi, :],
                                start=True,
                                stop=True,
                            )
                            e = expp.tile([128, 512], f16, tag="expP", name="expP")
                            nc.scalar.activation(
                                e[:], qk[:], Exp, bias=mb[:, jt : jt + 1], scale=1.0
                            )
                            expP[h][jt] = e
                    for h in range(2):
                        s = 2 * p + h
                        U = [psU.tile([128, 129], f32, tag="U", name="U") for _ in range(ICN)]
                        for jt in range(JT):
                            for ic in range(ICN):
                                nc.tensor.matmul(
                                    U[ic][:],
                                    expP[h][jt][:, ic * 128 : (ic + 1) * 128],
                                    rvaug[jt][:, 0:129],
                                    start=(jt == 0),
                                    stop=(jt == JT - 1),
                                )
                        # retrieval stage (row-land, all per-partition scalars)
                        Usb = []
                        for ic in range(ICN):
                            u = scrp.tile([128, 129], f32, tag="Usb", name="Usb")
                            nc.vector.tensor_copy(u[:], U[ic][:, 0:129])
                            Usb.append(u)
                        Bt = smallp.tile([128, 8], f32, tag="Bt", name="Bt")
                        for ic in range(ICN):
                            for r in range(R):
                                prod = scrp.tile([128, 64], f32, tag="prod", name="prod")
                                nc.vector.tensor_mul(
                                    prod[:],
                                    Usb[ic][:, r * 64 : (r + 1) * 64],
                                    rqW[ic][:, s * 64 : (s + 1) * 64],
                                )
                                nc.vector.tensor_reduce(
                                    Bt[:, r * 4 + ic : r * 4 + ic + 1],
                                    prod[:],
                                    axis=mybir.AxisListType.X,
                                    op=add,
                                )
                        lcol = smallp.tile([128, 4], f32, tag="lcol", name="lcol")
                        for ic in range(ICN):
                            nc.vector.tensor_copy(
                                lcol[:, ic : ic + 1], Usb[ic][:, 128:129]
                            )
                        linv = smallp.tile([128, 4], f32, tag="linv", name="linv")
                        nc.vector.reciprocal(linv[:], lcol[:])
                        dd = smallp.tile([128, 4], f32, tag="dd", name="dd")
                        nc.vector.tensor_sub(dd[:], Bt[:, 0:4], Bt[:, 4:8])
                        nc.vector.tensor_mul(dd[:], dd[:], linv[:])
                        g = smallp.tile([128, 4], f32, tag="g", name="g")
                        nc.scalar.activation(g[:], dd[:], Sigmoid)
                        w0 = smallp.tile([128, 4], f32, tag="w0", name="w0")
                        nc.vector.tensor_mul(w0[:], g[:], linv[:])
                        w1 = smallp.tile([128, 4], f32, tag="w1", name="w1")
                        nc.vector.tensor_sub(w1[:], linv[:], w0[:])
                        for ic in range(ICN):
                            v0 = scrp.tile([128, 64], f32, tag="v0", name="v0")
                            nc.vector.tensor_scalar_mul(
                                v0[:], Usb[ic][:, 0:64], w0[:, ic : ic + 1]
                            )
                            v1 = scrp.tile([128, 64], f32, tag="v1", name="v1")
                            nc.vector.tensor_scalar_mul(
                                v1[:], Usb[ic][:, 64:128], w1[:, ic : ic + 1]
                            )
                            nc.vector.tensor_add(
                                outcat[ic][:, s * 64 : (s + 1) * 64], v0[:], v1[:]
                            )

            # ============ Phase 3: output projection + 12-bit pack ============
            from concourse import bass_isa

            i32 = mybir.dt.int32
            shl = mybir.AluOpType.logical_shift_left
            maxop = mybir.AluOpType.max
            with (
                tc.tile_pool(name="octT", bufs=4) as octTp,
                tc.tile_pool(name="osb", bufs=4) as osbp,
                tc.tile_pool(name="packsc", bufs=8) as packscp,
                tc.tile_pool(name="packq", bufs=4) as packqp,
                tc.tile_pool(name="packo", bufs=8) as packop,
                tc.tile_pool(name="psT", bufs=2, space="PSUM") as psT,
            ):
                octT = [
                    octTp.tile([128, NSLICE], f16, tag="octT", name="octT") for _ in range(4)
                ]
                for ic in range(ICN):
                    for sc in range(4):
                        tp = psT.tile([128, 128], f32, tag="tp", name="tp")
                        nc.tensor.transpose(
                            tp[:],
                            outcat[ic][:, sc * 128 : (sc + 1) * 128],
                            identity[:],
                        )
                        nc.vector.tensor_copy(
                            octT[sc][:, ic * 128 : (ic + 1) * 128], tp[:]
                        )
                ot = []
                for ic in range(ICN):
                    o = osbp.tile([128, DIM], f32, tag="osb", name="osb")
                    for half in range(2):
                        ps = psA.tile([128, 512], f32, tag="psA", name="psA")
                        for sc in range(4):
                            nc.tensor.matmul(
                                ps[:],
                                octT[sc][:, ic * 128 : (ic + 1) * 128],
                                woutt[sc][:, half * 512 : (half + 1) * 512],
                                start=(sc == 0),
                                stop=(sc == 3),
                            )
                        nc.vector.tensor_copy(
                            o[:, half * 512 : (half + 1) * 512], ps[:]
                        )
                    ot.append(o)

                # per-core absmax -> scale
                m4 = packscp.tile([128, ICN], f32, tag="m4", name="m4")
                for ic in range(ICN):
                    nc.vector.tensor_reduce(
                        m4[:, ic : ic + 1], ot[ic][:],
                        axis=mybir.AxisListType.X, op=maxop,
                        apply_absolute_value=True,
                    )
                mg = packscp.tile([128, 1], f32, tag="mg", name="mg")
                nc.vector.tensor_reduce(
                    mg[:], m4[:], axis=mybir.AxisListType.X, op=maxop
                )
                gall = packscp.tile([128, 1], f32, tag="gall", name="gall")
                nc.gpsimd.partition_all_reduce(
                    gall[:], mg[:], channels=128, reduce_op=bass_isa.ReduceOp.absmax
                )
                nc.vector.tensor_scalar_max(gall[:], gall[:], 1e-30)
                osc = packscp.tile([128, 1], f32, tag="osc", name="osc")
                nc.vector.tensor_scalar(osc[:], gall[:], 1.0 / 2047.0, None, op0=mult)
                nc.sync.dma_start(oscd[0:1, 0:1], osc[0:1, 0:1])
                inv = packscp.tile([128, 1], f32, tag="inv", name="inv")
                nc.vector.reciprocal(inv[:], gall[:])
                invq = packscp.tile([128, 1], f32, tag="invq", name="invq")
                nc.vector.tensor_scalar(invq[:], inv[:], 2047.0, None, op0=mult)

                for ic in range(ICN):
                    qf = packqp.tile([128, DIM], f32, tag="qf", name="qf")
                    nc.vector.tensor_scalar(
                        qf[:], ot[ic][:], invq[:, 0:1], 2048.0, op0=mult, op1=add
                    )
                    qi = packqp.tile([128, DIM], i32, tag="qi", name="qi")
                    nc.vector.tensor_copy(qi[:], qf[:])
                    pk = packop.tile([128, OPACK], u8, tag="pk", name="pk")
                    his = []
                    for half in range(2):
                        v = qi[:, half * 512 : (half + 1) * 512]
                        lo = packop.tile([128, 512], i32, tag="lo", name="lo")
                        nc.vector.tensor_scalar(lo[:], v, 255, None, op0=band)
                        nc.vector.tensor_copy(
                            pk[:, half * 1024 : half * 1024 + 512], lo[:]
                        )
                        hi = packop.tile([128, 512], i32, tag="hi", name="hi")
                        nc.vector.tensor_scalar(hi[:], v, 8, None, op0=shr)
                        his.append(hi)
                    nc.vector.tensor_scalar(his[1][:], his[1][:], 4, None, op0=shl)
                    nc.vector.tensor_add(his[0][:], his[0][:], his[1][:])
                    nc.vector.tensor_copy(pk[:, 512:1024], his[0][:])
                    nc.sync.dma_start(
                        outd[ic * 128 : (ic + 1) * 128, :], pk[:]
                    )

    nc.compile()
    return nc


def _prep_in_maps(x, mask, Wsq, Wsk, Wrv, Wrq, Wrk, Wout):
    x = np.asarray(x, dtype=np.float32)
    mask = np.asarray(mask)
    Wsq = np.asarray(Wsq, dtype=np.float32)
    Wsk = np.asarray(Wsk, dtype=np.float32)
    Wrv = np.asarray(Wrv, dtype=np.float32)
    Wrq = np.asarray(Wrq, dtype=np.float32)
    Wrk = np.asarray(Wrk, dtype=np.float32)
    Wout = np.asarray(Wout, dtype=np.float32)

    # 12-bit quantization of x; the scale is folded into every weight that
    # multiplies x (device reconstructs x_int/2048, so weights carry
    # qscale*2048 ~= amax).
    amax = float(np.abs(x).max())
    qscale = np.float32(max(amax, 1e-30) / 2047.0)
    wscale = np.float32(qscale * 2048.0)

    wsq_eff = (Wsq * (np.float32(SCALE) * wscale)).astype(np.float16)
    # rqW = x @ wrq_eff where wrq_eff per head s: SCALE * Wrq_s @ Wrk^T
    wrq_eff = np.empty_like(Wrq)
    for s in range(S):
        wrq_eff[:, s * DH : (s + 1) * DH] = (
            Wrq[:, s * DH : (s + 1) * DH] @ Wrk.T
        ) * (np.float32(SCALE) * wscale)
    wrq_eff = wrq_eff.astype(np.float16)
    wsk16 = (Wsk * wscale).astype(np.float16)
    wrv16 = (Wrv * wscale).astype(np.float16)
    wout16 = Wout.astype(np.float16)
    mb = np.where(mask, np.float32(0.0), np.float32(-1e30)).astype(np.float32)

    xTb = [np.ascontiguousarray(x[b].T) for b in range(B)]

    def pack12(xq):
        q = (
            np.clip(np.round(xq / qscale), -2048, 2047).astype(np.int16) + 2048
        ).astype(np.uint16)
        v0, v1 = q[:, : NSLICE // 2], q[:, NSLICE // 2 :]
        A = (v0 & 255).astype(np.uint8)
        C = (v1 & 255).astype(np.uint8)
        Bp = ((v0 >> 8) | ((v1 >> 8) << 4)).astype(np.uint8)
        return np.ascontiguousarray(np.concatenate([A, Bp, C], axis=1))

    in_maps = []
    for c in range(NCORES):
        bc, isl = c // 4, c % 4
        in_maps.append(
            {
                "xq": pack12(xTb[bc][:, isl * NSLICE : (isl + 1) * NSLICE]),
                "mb": mb[bc],
                "wsq": np.ascontiguousarray(wsq_eff[c * WSHARD : (c + 1) * WSHARD, :]),
                "wsk": np.ascontiguousarray(wsk16[c * WSHARD : (c + 1) * WSHARD, :]),
                "wrq": np.ascontiguousarray(wrq_eff[c * WSHARD : (c + 1) * WSHARD, :]),
                "wrv": np.ascontiguousarray(wrv16[c * WSHARD : (c + 1) * WSHARD, :]),
                "wout": np.ascontiguousarray(wout16[c * WOSHARD : (c + 1) * WOSHARD, :]),
            }
        )
    return in_maps


def _get_nc():
    if "nc" not in _cache:
        _cache["nc"] = _build_program()
    return _cache["nc"]


def _get_runner():
    """Build the jitted SPMD executable once and cache it.

    Replicates bass2jax.run_bass_via_pjrt's lowering (same _bass_exec_p
    custom call, same donated-zero-output mechanism, same shard_map
    layout), but keeps the jitted function so repeat calls skip the
    ~3s re-trace/re-compile that run_bass_via_pjrt pays every time.
    """
    if "runner" in _cache:
        return _cache["runner"]

    import jax
    from jax.experimental.shard_map import shard_map
    from jax.sharding import Mesh, PartitionSpec
    from concourse import bass2jax, mybir
    from concourse.bass2jax import _bass_exec_p, install_neuronx_cc_hook, partition_id_tensor

    install_neuronx_cc_hook()
    nc = _get_nc()
    assert nc.dbg_addr is None or not nc.dbg_callbacks

    partition_name = nc.partition_id_tensor.name if nc.partition_id_tensor else None

    in_names = []
    out_names = []
    out_avals = []
    zero_shapes = []
    for alloc in nc.m.functions[0].allocations:
        if not isinstance(alloc, mybir.MemoryLocationSet):
            continue
        name = alloc.memorylocations[0].name
        if alloc.kind == "ExternalInput":
            if name != partition_name:
                in_names.append(name)
        elif alloc.kind == "ExternalOutput":
            shape = tuple(alloc.tensor_shape)
            dtype = mybir.dt.np(alloc.dtype)
            out_names.append(name)
            out_avals.append(jax.core.ShapedArray(shape, dtype))
            zero_shapes.append((shape, dtype))
    n_params = len(in_names)
    n_outs = len(out_avals)
    all_in_names = list(in_names) + list(out_names)
    if partition_name is not None:
        all_in_names.append(partition_name)

    extra_zero = None
    if nc.dbg_addr is not None:
        extra_zero = nc.dbg_addr.name

    donate = tuple(range(n_params, n_params + n_outs))

    def _body(*args):
        operands = list(args)
        if partition_name is not None:
            operands.append(partition_id_tensor())
        outs = _bass_exec_p.bind(
            *operands,
            out_avals=tuple(out_avals),
            in_names=tuple(all_in_names),
            out_names=tuple(out_names),
            lowering_input_output_aliases=(),
            sim_require_finite=True,
            sim_require_nnan=True,
            nc=nc,
        )
        return tuple(outs)

    devices = jax.devices()[:NCORES]
    assert len(devices) == NCORES
    mesh = Mesh(np.asarray(devices), ("core",))
    in_specs = (PartitionSpec("core"),) * (n_params + n_outs)
    out_specs = (PartitionSpec("core"),) * n_outs
    sharded = jax.jit(
        shard_map(
            _body, mesh=mesh, in_specs=in_specs, out_specs=out_specs, check_rep=False
        ),
        donate_argnums=donate,
        keep_unused=True,
    )
    runner = {
        "sharded": sharded,
        "in_names": in_names,
        "out_names": out_names,
        "zero_shapes": zero_shapes,
        "n_params": n_params,
        "extra_zero": extra_zero,
        "donation": None,
    }
    _cache["runner"] = runner
    return runner


def _run(in_maps):
    st = _get_runner()
    if st["extra_zero"] is not None:
        in_maps = [
            {**m, st["extra_zero"]: np.zeros((1, 2), np.uint32)} for m in in_maps
        ]
    concat_in = [
        np.concatenate([np.asarray(m[name]) for m in in_maps], axis=0)
        for name in st["in_names"]
    ]
    donation = st["donation"]
    if donation is None:
        donation = [
            np.zeros((NCORES * shape[0], *shape[1:]), dtype)
            for shape, dtype in st["zero_shapes"]
        ]
    out_arrs = st["sharded"](*concat_in, *donation)
    raw = {
        name: np.asarray(out_arrs[i]).reshape(NCORES, *st["zero_shapes"][i][0])
        for i, name in enumerate(st["out_names"])
    }
    # Recycle the output buffers as next call's donated outputs: the kernel
    # fully overwrites them, and reusing device-resident arrays avoids
    # re-uploading zero buffers over the tunnel every call.
    st["donation"] = list(out_arrs)
    results = [
        {"out": _unpack_out(raw["out"][c], raw["osc"][c])}
        for c in range(NCORES)
    ]
    return results


def _unpack_out(pk, osc):
    # 12-bit planar output: v0 = cols 0:512, v1 = cols 512:1024.
    osc = np.float32(np.asarray(osc).reshape(-1)[0])
    A = pk[:, 0:512].astype(np.int32)
    Bp = pk[:, 512:1024].astype(np.int32)
    C = pk[:, 1024:1536].astype(np.int32)
    o = np.empty((NSLICE, DIM), np.float32)
    o[:, 0:512] = A | ((Bp & 15) << 8)
    o[:, 512:1024] = C | ((Bp >> 4) << 8)
    o -= np.float32(2048.0)
    o *= osc
    return o


def kernel(**inputs):
    in_maps = _prep_in_maps(
        inputs["x"],
        inputs["mask"],
        inputs["Wsq"],
        inputs["Wsk"],
        inputs["Wrv"],
        inputs["Wrq"],
        inputs["Wrk"],
        inputs["Wout"],
    )
    results = _run(in_maps)
    out = np.empty((B, N, DIM), dtype=np.float32)
    for c in range(NCORES):
        bc, isl = c // 4, c % 4
        out[bc, isl * NSLICE : (isl + 1) * NSLICE, :] = results[c]["out"].astype(
            np.float32
        )
    return out


# revision 18
# speedup vs baseline: 1.1897x; 1.1897x over previous
"""CompositionalAttention TRN2 kernel.

Full (unsharded) inputs in, full output out.  Internally: 8 NeuronCores,
data-parallel over batch (4 cores per batch element) x parallel over query
rows (512 rows per core, all 8 search heads per core).

The axon tunnel to the cores is ~45 MB/s, so the kernel minimizes
host<->device traffic: each core uploads only a unique 1/8 shard of the
problem (its query-block of x^T in fp16, a 1/8 row-shard of each weight in
fp16) and the full per-batch data is reconstructed ON DEVICE with
AllGather collectives (NeuronLink is fast).  Output is downloaded as fp16.
The PJRT executable is built once and cached; output buffers are recycled
as donated inputs so no zero-buffer upload is paid per call.

Math (per batch b, search head s, query row i):
  sq = (x @ Wsq) * sc ; sk = x @ Wsk          (per head, d=64)
  P  = softmax_j(sq_i . sk_j)                 (n x n attention)
  U_r = P @ rv_r                              (rv = x @ Wrv, r=0,1)
  retrieved_r = U_r / l,  l = sum_j expP
  sim_r = rq . (retrieved_r @ Wrk) = rowdot(U_r, rq @ Wrk^T) / l
  attn = softmax_r(sim)  ==  sigmoid(sim_0 - sim_1) for r=2 (exact)
  out_s = attn*retrieved_0 + (1-attn)*retrieved_1
  out = concat_s(out_s) @ Wout

Host folds: scale into Wsq; Wrk into Wrq (rqW = x @ (sc * Wrq_s @ Wrk^T));
mask into an additive exp bias.  exp is computed without max-subtraction
(sim ~ N(0,1), max |sim| << 80, so fp16 exp inputs are safe).
"""

import sys

sys.path.insert(0, "/opt/trn_rl_repo")

import numpy as np

B, N, DIM, S, R, DH = 2, 2048, 1024, 8, 2, 64
SD, RD = S * DH, R * DH  # 512, 128
NCORES = 8
NSLICE = N // 4  # 512 query rows per core
SCALE = DH**-0.5
KT = DIM // 128  # 8 contraction tiles
JT = N // 128  # 16 key tiles
ICN = NSLICE // 128  # 4 query chunks
PAIRS = S // 2
WSHARD = DIM // NCORES  # 128 rows of each [DIM, .] weight per core
WOSHARD = SD // NCORES  # 64 rows of Wout per core

_cache = {}


def _build_program():
    import concourse.bass as bass
    import concourse.tile as tile
    from concourse import bacc, mybir
    from concourse.masks import make_identity

    f32 = mybir.dt.float32
    f16 = mybir.dt.float16
    u8 = mybir.dt.uint8
    Exp = mybir.ActivationFunctionType.Exp
    Sigmoid = mybir.ActivationFunctionType.Sigmoid
    add = mybir.AluOpType.add
    mult = mybir.AluOpType.mult
    band = mybir.AluOpType.bitwise_and
    shr = mybir.AluOpType.logical_shift_right

    nc = bacc.Bacc(
        "TRN2", target_bir_lowering=False, debug=False, num_devices=NCORES
    )

    # Per-core unique inputs (x packed to 12 bits, weights fp16, to cut
    # tunnel bytes).
    # xq: this core's query block of x^T, i.e. xT[:, isl*512:(isl+1)*512],
    # quantized to 12-bit fixed point and packed as 3 byte-planes
    # [A | B | C] where token u pairs with token u+256:
    #   v0 = A + 256*(B & 15),  v1 = C + 256*(B >> 4),  x = (v - 2048)/2048
    # (the dynamic quantization scale is folded into the weights on host).
    PACK = 3 * NSLICE // 2  # 768 packed bytes per row
    xqd = nc.dram_tensor("xq", [DIM, PACK], u8, kind="ExternalInput").ap()
    mbd = nc.dram_tensor("mb", [N], f32, kind="ExternalInput").ap()
    wsqd = nc.dram_tensor("wsq", [WSHARD, SD], f16, kind="ExternalInput").ap()
    wskd = nc.dram_tensor("wsk", [WSHARD, SD], f16, kind="ExternalInput").ap()
    wrqd = nc.dram_tensor("wrq", [WSHARD, SD], f16, kind="ExternalInput").ap()
    wrvd = nc.dram_tensor("wrv", [WSHARD, RD], f16, kind="ExternalInput").ap()
    woutd = nc.dram_tensor("wout", [WOSHARD, DIM], f16, kind="ExternalInput").ap()
    # Output is packed the same way as x: 12-bit planar [A | B | C] along
    # columns (v0 = cols 0:512, v1 = cols 512:1024 of the f32 output), with
    # a per-core dynamic scale in osc: out = (v - 2048) * osc.
    # Row NSLICE carries the f32 scale in its first 4 bytes (a separate tiny
    # output tensor would cost an extra ~45ms D2H round-trip per call).
    OPACK = 3 * DIM // 2  # 1536 packed bytes per row
    outd = nc.dram_tensor("out", [NSLICE + 1, OPACK], u8, kind="ExternalOutput").ap()

    GROUPS_X = [[0, 1, 2, 3], [4, 5, 6, 7]]
    GROUPS_W = [list(range(NCORES))]

    with tile.TileContext(nc) as tc:
        with (
            tc.tile_pool(name="gdram", bufs=16, space="DRAM") as gdram,
            tc.tile_pool(name="sk", bufs=4) as skp,
            tc.tile_pool(name="sq", bufs=4) as sqp,
            tc.tile_pool(name="rqw", bufs=4) as rqwp,
            tc.tile_pool(name="rvaug", bufs=JT) as rvap,
            tc.tile_pool(name="consts", bufs=4) as constp,
            tc.tile_pool(name="outcat", bufs=4) as outcatp,
            tc.tile_pool(name="woutp", bufs=4) as woutp,
            tc.tile_pool(name="psA", bufs=2, space="PSUM") as psA,
        ):
            # ============ Phase 0: on-device allgather of shards ============
            # x: gather the 4 query-blocks of this batch group (still packed).
            # xg layout: block g rows [g*1024,(g+1)*1024) = packed
            # xT[:, g*512:(g+1)*512]
            xq_b = gdram.tile([DIM, PACK], u8, name="xq_b")
            xg = gdram.tile([4 * DIM, PACK], u8, name="xg")
            nc.gpsimd.dma_start(xq_b[:], xqd[:])
            nc.gpsimd.collective_compute(
                "AllGather",
                mybir.AluOpType.bypass,
                replica_groups=GROUPS_X,
                ins=[xq_b.opt()],
                outs=[xg.opt()],
            )

            def gather_w(name, ind, shard_rows, cols):
                b = gdram.tile([shard_rows, cols], f16, name=name + "_b")
                g = gdram.tile(
                    [NCORES * shard_rows, cols], f16, name=name + "_g",
                    addr_space="Shared",
                )
                nc.gpsimd.dma_start(b[:], ind[:])
                nc.gpsimd.collective_compute(
                    "AllGather",
                    mybir.AluOpType.bypass,
                    replica_groups=GROUPS_W,
                    ins=[b.opt()],
                    outs=[g.opt()],
                )
                return g

            wsq_g = gather_w("wsq", wsqd, WSHARD, SD)
            wsk_g = gather_w("wsk", wskd, WSHARD, SD)
            wrq_g = gather_w("wrq", wrqd, WSHARD, SD)
            wrv_g = gather_w("wrv", wrvd, WSHARD, RD)
            wout_g = gather_w("wout", woutd, WOSHARD, DIM)

            # --- constants ---
            mb = constp.tile([128, JT], f32, tag="mb", name="mb")
            nc.sync.dma_start(mb[:], mbd.rearrange("(t p) -> p t", p=128))
            identity = constp.tile([128, 128], f32, tag="ident", name="ident")
            make_identity(nc, identity[:])

            skT = [skp.tile([128, N], f16, tag="skT", name="skT") for _ in range(4)]
            sqT = [sqp.tile([128, NSLICE], f16, tag="sqT", name="sqT") for _ in range(4)]
            rqW = [rqwp.tile([128, SD], f32, tag="rqW", name="rqW") for _ in range(4)]
            rvaug = [rvap.tile([128, 132], f16, tag="rvaug", name="rvaug") for _ in range(JT)]

            # ============ Phase 1: projections ============
            with (
                tc.tile_pool(name="xt", bufs=KT) as xtp,
                tc.tile_pool(name="xtq", bufs=KT) as xtqp,
                tc.tile_pool(name="unp", bufs=10) as unp,
                tc.tile_pool(name="wl", bufs=12) as wlp,
                tc.tile_pool(name="wrq", bufs=KT) as wrqp,
                tc.tile_pool(name="rvbf", bufs=1) as rvbfp,
            ):
                H = NSLICE // 2  # 256 tokens per unpacked plane

                def unpack_block(dst, dst_col, src, src_row):
                    # src rows [src_row, src_row+128) hold a packed
                    # [128, 768] block; writes 512 f16 tokens at
                    # dst[:, dst_col : dst_col+512].
                    P = unp.tile([128, PACK], u8, tag="P", name="P")
                    nc.sync.dma_start(P[:], src[src_row : src_row + 128, :])
                    af = unp.tile([128, H], f32, tag="af", name="af")
                    nc.vector.tensor_copy(af[:], P[:, 0:H])
                    cf = unp.tile([128, H], f32, tag="cf", name="cf")
                    nc.vector.tensor_copy(cf[:], P[:, 2 * H : 3 * H])
                    b15u = unp.tile([128, H], u8, tag="b15u", name="b15u")
                    nc.vector.tensor_scalar(b15u[:], P[:, H : 2 * H], 15, None, op0=band)
                    b15 = unp.tile([128, H], f32, tag="b15", name="b15")
                    nc.vector.tensor_copy(b15[:], b15u[:])
                    bhiu = unp.tile([128, H], u8, tag="bhiu", name="bhiu")
                    nc.vector.tensor_scalar(bhiu[:], P[:, H : 2 * H], 4, None, op0=shr)
                    bhi = unp.tile([128, H], f32, tag="bhi", name="bhi")
                    nc.vector.tensor_copy(bhi[:], bhiu[:])
                    v0 = unp.tile([128, H], f32, tag="v0", name="v0")
                    nc.vector.tensor_scalar(v0[:], b15[:], 256.0, None, op0=mult)
                    nc.vector.tensor_add(v0[:], v0[:], af[:])
                    v1 = unp.tile([128, H], f32, tag="v1", name="v1")
                    nc.vector.tensor_scalar(v1[:], bhi[:], 256.0, None, op0=mult)
                    nc.vector.tensor_add(v1[:], v1[:], cf[:])
                    nc.vector.tensor_scalar(
                        dst[:, dst_col : dst_col + H],
                        v0[:], -2048.0, 1.0 / 2048.0, op0=add, op1=mult,
                    )
                    nc.vector.tensor_scalar(
                        dst[:, dst_col + H : dst_col + 2 * H],
                        v1[:], -2048.0, 1.0 / 2048.0, op0=add, op1=mult,
                    )

                xt = []
                xtq = []
                for kt in range(KT):
                    t = xtp.tile([128, N], f16, tag="xt", name="xt")
                    for g in range(4):
                        unpack_block(t, g * NSLICE, xg, g * DIM + kt * 128)
                    xt.append(t)
                    tq = xtqp.tile([128, NSLICE], f16, tag="xtq", name="xtq")
                    unpack_block(tq, 0, xqd, kt * 128)
                    xtq.append(tq)
                wrqt = []
                for kt in range(KT):
                    t = wrqp.tile([128, SD], f16, tag="wrq", name="wrq")
                    nc.sync.dma_start(t[:], wrq_g[kt * 128 : (kt + 1) * 128, :])
                    wrqt.append(t)

                # skT[dt] = (Wsk[:, dt]).T-proj of x: [128 d, 2048 j]
                for dt in range(4):
                    wk = []
                    for kt in range(KT):
                        t = wlp.tile([128, 128], f16, tag="wl", name="wl")
                        nc.sync.dma_start(
                            t[:],
                            wsk_g[kt * 128 : (kt + 1) * 128, dt * 128 : (dt + 1) * 128],
                        )
                        wk.append(t)
                    for jc in range(4):
                        ps = psA.tile([128, 512], f32, tag="psA", name="psA")
                        for kt in range(KT):
                            nc.tensor.matmul(
                                ps[:],
                                wk[kt][:],
                                xt[kt][:, jc * 512 : (jc + 1) * 512],
                                start=(kt == 0),
                                stop=(kt == KT - 1),
                            )
                        nc.vector.tensor_copy(
                            skT[dt][:, jc * 512 : (jc + 1) * 512], ps[:]
                        )

                # sqT[dt]: [128 d, 512 i] (scale pre-folded into Wsq)
                for dt in range(4):
                    wk = []
                    for kt in range(KT):
                        t = wlp.tile([128, 128], f16, tag="wl", name="wl")
                        nc.sync.dma_start(
                            t[:],
                            wsq_g[kt * 128 : (kt + 1) * 128, dt * 128 : (dt + 1) * 128],
                        )
                        wk.append(t)
                    ps = psA.tile([128, 512], f32, tag="psA", name="psA")
                    for kt in range(KT):
                        nc.tensor.matmul(
                            ps[:],
                            wk[kt][:],
                            xtq[kt][:],
                            start=(kt == 0),
                            stop=(kt == KT - 1),
                        )
                    nc.vector.tensor_copy(sqT[dt][:], ps[:])

                # rqW[ic]: row-land [128 i, 512 sd] = x_i @ (sc*Wrq_s@Wrk^T)
                for ic in range(ICN):
                    ps = psA.tile([128, 512], f32, tag="psA", name="psA")
                    for kt in range(KT):
                        nc.tensor.matmul(
                            ps[:],
                            xtq[kt][:, ic * 128 : (ic + 1) * 128],
                            wrqt[kt][:],
                            start=(kt == 0),
                            stop=(kt == KT - 1),
                        )
                    nc.vector.tensor_copy(rqW[ic][:], ps[:])

                # rvT [128 d, 2048 j] -> transpose to rv_aug [j, 132] (f16)
                rvbf = rvbfp.tile([128, N], f32, tag="rvbf", name="rvbf")
                wrvt = []
                for kt in range(KT):
                    t = wlp.tile([128, 128], f16, tag="wl", name="wl")
                    nc.sync.dma_start(t[:], wrv_g[kt * 128 : (kt + 1) * 128, :])
                    wrvt.append(t)
                for jc in range(4):
                    ps = psA.tile([128, 512], f32, tag="psA", name="psA")
                    for kt in range(KT):
                        nc.tensor.matmul(
                            ps[:],
                            wrvt[kt][:],
                            xt[kt][:, jc * 512 : (jc + 1) * 512],
                            start=(kt == 0),
                            stop=(kt == KT - 1),
                        )
                    nc.vector.tensor_copy(rvbf[:, jc * 512 : (jc + 1) * 512], ps[:])
                for jt in range(JT):
                    nc.gpsimd.memset(rvaug[jt][:], 1.0)
                for g in range(4):
                    ps = psA.tile([128, 512], f32, tag="psA", name="psA")
                    for k in range(4):
                        jt = g * 4 + k
                        nc.tensor.transpose(
                            ps[:, k * 128 : (k + 1) * 128],
                            rvbf[:, jt * 128 : (jt + 1) * 128],
                            identity[:],
                        )
                    for k in range(4):
                        jt = g * 4 + k
                        nc.vector.tensor_copy(
                            rvaug[jt][:, 0:128], ps[:, k * 128 : (k + 1) * 128]
                        )

            # ============ Phase 2: attention + retrieval ============
            woutt = []
            for sc in range(4):
                t = woutp.tile([128, DIM], f16, tag="wout", name="wout")
                nc.sync.dma_start(t[:], wout_g[sc * 128 : (sc + 1) * 128, :])
                woutt.append(t)

            outcat = [outcatp.tile([128, SD], f32, tag="outcat", name="outcat") for _ in range(4)]

            with (
                tc.tile_pool(name="expp", bufs=36) as expp,
                tc.tile_pool(name="small", bufs=16) as smallp,
                tc.tile_pool(name="scr", bufs=4) as scrp,
                tc.tile_pool(name="psQK", bufs=2, space="PSUM") as psQK,
                tc.tile_pool(name="psU", bufs=4, space="PSUM") as psU,
            ):
                for p in range(PAIRS):
                    expP = [[None] * JT, [None] * JT]
                    for jt in range(JT):
                        for h in range(2):
                            qk = psQK.tile([128, 512], f32, tag="qk", name="qk")
                            lo, hi = h * 64, (h + 1) * 64
                            nc.tensor.matmul(
                                qk[:],
                                skT[p][lo:hi, jt * 128 : (jt + 1) * 128],
                                sqT[p][lo:hi, :],
                                start=True,
                                stop=True,
                            )
                            e = expp.tile([128, 512], f16, tag="expP", name="expP")
                            nc.scalar.activation(
                                e[:], qk[:], Exp, bias=mb[:, jt : jt + 1], scale=1.0
                            )
                            expP[h][jt] = e
                    for h in range(2):
                        s = 2 * p + h
                        U = [psU.tile([128, 129], f32, tag="U", name="U") for _ in range(ICN)]
                        for jt in range(JT):
                            for ic in range(ICN):
                                nc.tensor.matmul(
                                    U[ic][:],
                                    expP[h][jt][:, ic * 128 : (ic + 1) * 128],
                                    rvaug[jt][:, 0:129],
                                    start=(jt == 0),
                                    stop=(jt == JT - 1),
                                )
                        # retrieval stage (row-land, all per-partition scalars)
                        Usb = []
                        for ic in range(ICN):
                            u = scrp.tile([128, 129], f32, tag="Usb", name="Usb")
                            nc.vector.tensor_copy(u[:], U[ic][:, 0:129])
                            Usb.append(u)
                        Bt = smallp.tile([128, 8], f32, tag="Bt", name="Bt")
                        for ic in range(ICN):
                            for r in range(R):
                                prod = scrp.tile([128, 64], f32, tag="prod", name="prod")
                                nc.vector.tensor_mul(
                                    prod[:],
                                    Usb[ic][:, r * 64 : (r + 1) * 64],
                                    rqW[ic][:, s * 64 : (s + 1) * 64],
                                )
                                nc.vector.tensor_reduce(
                                    Bt[:, r * 4 + ic : r * 4 + ic + 1],
                                    prod[:],
                                    axis=mybir.AxisListType.X,
                                    op=add,
                                )
                        lcol = smallp.tile([128, 4], f32, tag="lcol", name="lcol")
                        for ic in range(ICN):
                            nc.vector.tensor_copy(
                                lcol[:, ic : ic + 1], Usb[ic][:, 128:129]
                            )
                        linv = smallp.tile([128, 4], f32, tag="linv", name="linv")
                        nc.vector.reciprocal(linv[:], lcol[:])
                        dd = smallp.tile([128, 4], f32, tag="dd", name="dd")
                        nc.vector.tensor_sub(dd[:], Bt[:, 0:4], Bt[:, 4:8])
                        nc.vector.tensor_mul(dd[:], dd[:], linv[:])
                        g = smallp.tile([128, 4], f32, tag="g", name="g")
                        nc.scalar.activation(g[:], dd[:], Sigmoid)
                        w0 = smallp.tile([128, 4], f32, tag="w0", name="w0")
                        nc.vector.tensor_mul(w0[:], g[:], linv[:])
                        w1 = smallp.tile([128, 4], f32, tag="w1", name="w1")
                        nc.vector.tensor_sub(w1[:], linv[:], w0[:])
                        for ic in range(ICN):
                            v0 = scrp.tile([128, 64], f32, tag="v0", name="v0")
                            nc.vector.tensor_scalar_mul(
                                v0[:], Usb[ic][:, 0:64], w0[:, ic : ic + 1]
                            )
                            v1 = scrp.tile([128, 64], f32, tag="v1", name="v1")
                            nc.vector.tensor_scalar_mul(
                                v1[:], Usb[ic][:, 64:128], w1[:, ic : ic + 1]
                            )
                            nc.vector.tensor_add(
                                outcat[ic][:, s * 64 : (s + 1) * 64], v0[:], v1[:]
                            )

            # ============ Phase 3: output projection + 12-bit pack ============
            from concourse import bass_isa

            i32 = mybir.dt.int32
            shl = mybir.AluOpType.logical_shift_left
            maxop = mybir.AluOpType.max
            with (
                tc.tile_pool(name="octT", bufs=4) as octTp,
                tc.tile_pool(name="osb", bufs=4) as osbp,
                tc.tile_pool(name="packsc", bufs=8) as packscp,
                tc.tile_pool(name="packq", bufs=4) as packqp,
                tc.tile_pool(name="packo", bufs=8) as packop,
                tc.tile_pool(name="psT", bufs=2, space="PSUM") as psT,
            ):
                octT = [
                    octTp.tile([128, NSLICE], f16, tag="octT", name="octT") for _ in range(4)
                ]
                for ic in range(ICN):
                    for sc in range(4):
                        tp = psT.tile([128, 128], f32, tag="tp", name="tp")
                        nc.tensor.transpose(
                            tp[:],
                            outcat[ic][:, sc * 128 : (sc + 1) * 128],
                            identity[:],
                        )
                        nc.vector.tensor_copy(
                            octT[sc][:, ic * 128 : (ic + 1) * 128], tp[:]
                        )
                ot = []
                for ic in range(ICN):
                    o = osbp.tile([128, DIM], f32, tag="osb", name="osb")
                    for half in range(2):
                        ps = psA.tile([128, 512], f32, tag="psA", name="psA")
                        for sc in range(4):
                            nc.tensor.matmul(
                                ps[:],
                                octT[sc][:, ic * 128 : (ic + 1) * 128],
                                woutt[sc][:, half * 512 : (half + 1) * 512],
                                start=(sc == 0),
                                stop=(sc == 3),
                            )
                        nc.vector.tensor_copy(
                            o[:, half * 512 : (half + 1) * 512], ps[:]
                        )
                    ot.append(o)

                # per-core absmax -> scale
                m4 = packscp.tile([128, ICN], f32, tag="m4", name="m4")
                for ic in range(ICN):
                    nc.vector.tensor_reduce(
                        m4[:, ic : ic + 1], ot[ic][:],
                        axis=mybir.AxisListType.X, op=maxop,
                        apply_absolute_value=True,
                    )
                mg = packscp.tile([128, 1], f32, tag="mg", name="mg")
                nc.vector.tensor_reduce(
                    mg[:], m4[:], axis=mybir.AxisListType.X, op=maxop
                )
                gall = packscp.tile([128, 1], f32, tag="gall", name="gall")
                nc.gpsimd.partition_all_reduce(
                    gall[:], mg[:], channels=128, reduce_op=bass_isa.ReduceOp.absmax
                )
                nc.vector.tensor_scalar_max(gall[:], gall[:], 1e-30)
                osc = packscp.tile([128, 1], f32, tag="osc", name="osc")
                nc.vector.tensor_scalar(osc[:], gall[:], 1.0 / 2047.0, None, op0=mult)
                nc.sync.dma_start(
                    outd[NSLICE : NSLICE + 1, 0:4], osc.bitcast(u8)[0:1, 0:4]
                )
                inv = packscp.tile([128, 1], f32, tag="inv", name="inv")
                nc.vector.reciprocal(inv[:], gall[:])
                invq = packscp.tile([128, 1], f32, tag="invq", name="invq")
                nc.vector.tensor_scalar(invq[:], inv[:], 2047.0, None, op0=mult)

                for ic in range(ICN):
                    qf = packqp.tile([128, DIM], f32, tag="qf", name="qf")
                    nc.vector.tensor_scalar(
                        qf[:], ot[ic][:], invq[:, 0:1], 2048.0, op0=mult, op1=add
                    )
                    qi = packqp.tile([128, DIM], i32, tag="qi", name="qi")
                    nc.vector.tensor_copy(qi[:], qf[:])
                    pk = packop.tile([128, OPACK], u8, tag="pk", name="pk")
                    his = []
                    for half in range(2):
                        v = qi[:, half * 512 : (half + 1) * 512]
                        lo = packop.tile([128, 512], i32, tag="lo", name="lo")
                        nc.vector.tensor_scalar(lo[:], v, 255, None, op0=band)
                        nc.vector.tensor_copy(
                            pk[:, half * 1024 : half * 1024 + 512], lo[:]
                        )
                        hi = packop.tile([128, 512], i32, tag="hi", name="hi")
                        nc.vector.tensor_scalar(hi[:], v, 8, None, op0=shr)
                        his.append(hi)
                    nc.vector.tensor_scalar(his[1][:], his[1][:], 4, None, op0=shl)
                    nc.vector.tensor_add(his[0][:], his[0][:], his[1][:])
                    nc.vector.tensor_copy(pk[:, 512:1024], his[0][:])
                    nc.sync.dma_start(
                        outd[ic * 128 : (ic + 1) * 128, :], pk[:]
                    )

    nc.compile()
    return nc


def _prep_in_maps(x, mask, Wsq, Wsk, Wrv, Wrq, Wrk, Wout):
    x = np.asarray(x, dtype=np.float32)
    mask = np.asarray(mask)
    Wsq = np.asarray(Wsq, dtype=np.float32)
    Wsk = np.asarray(Wsk, dtype=np.float32)
    Wrv = np.asarray(Wrv, dtype=np.float32)
    Wrq = np.asarray(Wrq, dtype=np.float32)
    Wrk = np.asarray(Wrk, dtype=np.float32)
    Wout = np.asarray(Wout, dtype=np.float32)

    # 12-bit quantization of x; the scale is folded into every weight that
    # multiplies x (device reconstructs x_int/2048, so weights carry
    # qscale*2048 ~= amax).
    amax = float(np.abs(x).max())
    qscale = np.float32(max(amax, 1e-30) / 2047.0)
    wscale = np.float32(qscale * 2048.0)

    wsq_eff = (Wsq * (np.float32(SCALE) * wscale)).astype(np.float16)
    # rqW = x @ wrq_eff where wrq_eff per head s: SCALE * Wrq_s @ Wrk^T
    wrq_eff = np.empty_like(Wrq)
    for s in range(S):
        wrq_eff[:, s * DH : (s + 1) * DH] = (
            Wrq[:, s * DH : (s + 1) * DH] @ Wrk.T
        ) * (np.float32(SCALE) * wscale)
    wrq_eff = wrq_eff.astype(np.float16)
    wsk16 = (Wsk * wscale).astype(np.float16)
    wrv16 = (Wrv * wscale).astype(np.float16)
    wout16 = Wout.astype(np.float16)
    mb = np.where(mask, np.float32(0.0), np.float32(-1e30)).astype(np.float32)

    xTb = [np.ascontiguousarray(x[b].T) for b in range(B)]

    def pack12(xq):
        q = (
            np.clip(np.round(xq / qscale), -2048, 2047).astype(np.int16) + 2048
        ).astype(np.uint16)
        v0, v1 = q[:, : NSLICE // 2], q[:, NSLICE // 2 :]
        A = (v0 & 255).astype(np.uint8)
        C = (v1 & 255).astype(np.uint8)
        Bp = ((v0 >> 8) | ((v1 >> 8) << 4)).astype(np.uint8)
        return np.ascontiguousarray(np.concatenate([A, Bp, C], axis=1))

    in_maps = []
    for c in range(NCORES):
        bc, isl = c // 4, c % 4
        in_maps.append(
            {
                "xq": pack12(xTb[bc][:, isl * NSLICE : (isl + 1) * NSLICE]),
                "mb": mb[bc],
                "wsq": np.ascontiguousarray(wsq_eff[c * WSHARD : (c + 1) * WSHARD, :]),
                "wsk": np.ascontiguousarray(wsk16[c * WSHARD : (c + 1) * WSHARD, :]),
                "wrq": np.ascontiguousarray(wrq_eff[c * WSHARD : (c + 1) * WSHARD, :]),
                "wrv": np.ascontiguousarray(wrv16[c * WSHARD : (c + 1) * WSHARD, :]),
                "wout": np.ascontiguousarray(wout16[c * WOSHARD : (c + 1) * WOSHARD, :]),
            }
        )
    return in_maps


def _get_nc():
    if "nc" not in _cache:
        _cache["nc"] = _build_program()
    return _cache["nc"]


def _get_runner():
    """Build the jitted SPMD executable once and cache it.

    Replicates bass2jax.run_bass_via_pjrt's lowering (same _bass_exec_p
    custom call, same donated-zero-output mechanism, same shard_map
    layout), but keeps the jitted function so repeat calls skip the
    ~3s re-trace/re-compile that run_bass_via_pjrt pays every time.
    """
    if "runner" in _cache:
        return _cache["runner"]

    import jax
    from jax.experimental.shard_map import shard_map
    from jax.sharding import Mesh, PartitionSpec
    from concourse import bass2jax, mybir
    from concourse.bass2jax import _bass_exec_p, install_neuronx_cc_hook, partition_id_tensor

    install_neuronx_cc_hook()
    nc = _get_nc()
    assert nc.dbg_addr is None or not nc.dbg_callbacks

    partition_name = nc.partition_id_tensor.name if nc.partition_id_tensor else None

    in_names = []
    out_names = []
    out_avals = []
    zero_shapes = []
    for alloc in nc.m.functions[0].allocations:
        if not isinstance(alloc, mybir.MemoryLocationSet):
            continue
        name = alloc.memorylocations[0].name
        if alloc.kind == "ExternalInput":
            if name != partition_name:
                in_names.append(name)
        elif alloc.kind == "ExternalOutput":
            shape = tuple(alloc.tensor_shape)
            dtype = mybir.dt.np(alloc.dtype)
            out_names.append(name)
            out_avals.append(jax.core.ShapedArray(shape, dtype))
            zero_shapes.append((shape, dtype))
    n_params = len(in_names)
    n_outs = len(out_avals)
    all_in_names = list(in_names) + list(out_names)
    if partition_name is not None:
        all_in_names.append(partition_name)

    extra_zero = None
    if nc.dbg_addr is not None:
        extra_zero = nc.dbg_addr.name

    donate = tuple(range(n_params, n_params + n_outs))

    def _body(*args):
        operands = list(args)
        if partition_name is not None:
            operands.append(partition_id_tensor())
        outs = _bass_exec_p.bind(
            *operands,
            out_avals=tuple(out_avals),
            in_names=tuple(all_in_names),
            out_names=tuple(out_names),
            lowering_input_output_aliases=(),
            sim_require_finite=True,
            sim_require_nnan=True,
            nc=nc,
        )
        return tuple(outs)

    devices = jax.devices()[:NCORES]
    assert len(devices) == NCORES
    mesh = Mesh(np.asarray(devices), ("core",))
    in_specs = (PartitionSpec("core"),) * (n_params + n_outs)
    out_specs = (PartitionSpec("core"),) * n_outs
    sharded = jax.jit(
        shard_map(
            _body, mesh=mesh, in_specs=in_specs, out_specs=out_specs, check_rep=False
        ),
        donate_argnums=donate,
        keep_unused=True,
    )
    runner = {
        "sharded": sharded,
        "in_names": in_names,
        "out_names": out_names,
        "zero_shapes": zero_shapes,
        "n_params": n_params,
        "extra_zero": extra_zero,
        "donation": None,
    }
    _cache["runner"] = runner
    return runner


def _run(in_maps):
    st = _get_runner()
    if st["extra_zero"] is not None:
        in_maps = [
            {**m, st["extra_zero"]: np.zeros((1, 2), np.uint32)} for m in in_maps
        ]
    concat_in = [
        np.concatenate([np.asarray(m[name]) for m in in_maps], axis=0)
        for name in st["in_names"]
    ]
    donation = st["donation"]
    if donation is None:
        donation = [
            np.zeros((NCORES * shape[0], *shape[1:]), dtype)
            for shape, dtype in st["zero_shapes"]
        ]
    out_arrs = st["sharded"](*concat_in, *donation)
    raw = {
        name: np.asarray(out_arrs[i]).reshape(NCORES, *st["zero_shapes"][i][0])
        for i, name in enumerate(st["out_names"])
    }
    # Recycle the output buffers as next call's donated outputs: the kernel
    # fully overwrites them, and reusing device-resident arrays avoids
    # re-uploading zero buffers over the tunnel every call.
    st["donation"] = list(out_arrs)
    results = [{"out": _unpack_out(raw["out"][c])} for c in range(NCORES)]
    return results


def _unpack_out(pkfull):
    # 12-bit planar output: v0 = cols 0:512, v1 = cols 512:1024; the scale
    # rides in the first 4 bytes of the extra row.
    osc = pkfull[NSLICE, 0:4].copy().view(np.float32)[0]
    pk = pkfull[:NSLICE]
    A = pk[:, 0:512].astype(np.int32)
    Bp = pk[:, 512:1024].astype(np.int32)
    C = pk[:, 1024:1536].astype(np.int32)
    o = np.empty((NSLICE, DIM), np.float32)
    o[:, 0:512] = A | ((Bp & 15) << 8)
    o[:, 512:1024] = C | ((Bp >> 4) << 8)
    o -= np.float32(2048.0)
    o *= osc
    return o


def kernel(**inputs):
    in_maps = _prep_in_maps(
        inputs["x"],
        inputs["mask"],
        inputs["Wsq"],
        inputs["Wsk"],
        inputs["Wrv"],
        inputs["Wrq"],
        inputs["Wrk"],
        inputs["Wout"],
    )
    results = _run(in_maps)
    out = np.empty((B, N, DIM), dtype=np.float32)
    for c in range(NCORES):
        bc, isl = c // 4, c % 4
        out[bc, isl * NSLICE : (isl + 1) * NSLICE, :] = results[c]["out"].astype(
            np.float32
        )
    return out
